# revision 21
# baseline (speedup 1.0000x reference)
"""PointNet Set Abstraction on 8 Trainium2 NeuronCores.

Sharding: data-parallel over batch B=8, one point cloud per core.

Per-core device pipeline:
  - FPS (1023 serial iterations) on DVE (custom fused ops) + gpsimd
    partition reduces. Produces the 1024 sampled points directly (no
    index extraction needed).
  - Pointwise MLP (3x conv1x1+BN+ReLU) over all N=8192 input points on
    PE/ACT (overlapped with FPS), transposed and staged to DRAM as
    GT [8192, 256].
  - kNN scores via K=5 PE matmul (s = 2 c.q - |q|^2 - |c|^2, monotone
    in -distance), exact top-32 per centroid row via vector max8 /
    max_index / match_replace rounds (ties resolved to lowest index,
    matching lax.top_k).
  - Feature gather: dma_gather of 4x8192 rows of 1KB from GT, max-pool
    over the 32 neighbors with strided tensor_reduce.

Host only reshapes/transposes per-core outputs into the full result.
"""

import numpy as np

B, N, S, K = 8, 8192, 1024, 32
IN_CH = 64
MLP_CH = [128, 128, 256]
EPS = 1e-5
FPS_ITERS = S - 1  # serial FPS steps after seeding index 0
NB = N // 128  # 64 free elems per partition in [128, 64] point tiles
SBLK = S // 128  # 8 centroid blocks
NCALLS = 4  # dma_gather calls (8 neighbor slots each)
RPC = K // NCALLS  # 8 r-slots per call
IMM_NEG = -3.0e38

_BUILD_CACHE = {}


# --------------------------------------------------------------------------
# Custom DVE ops (registered at runtime; self-pinned shas)
# --------------------------------------------------------------------------
def _register_dve_ops():
    import concourse.dve_ops as dve_ops
    from concourse.dve_ops import DveOp, OPS, CUSTOM_DVE_SPECS, _SUB_OPCODE_FOR_NAME
    from concourse.dve_spec import (
        Spec, Src0, Src1, C0, C1, Zero, AluOp, sq, minn, select, eq, lower,
        _has_src1,
    )
    from concourse.dve_uop import DveOpSpec

    if "PN_SQ2" in _SUB_OPCODE_FOR_NAME:
        from concourse.dve_ops import OPS as _O
        return {op.name: op for op in _O if op.name.startswith("PN_")}

    def ref_sq2(in0, in1, s0, s1, imm2):
        a = (in0 - s0).astype(np.float32)
        b = (in1 - s1).astype(np.float32)
        return (a * a + b * b).astype(np.float32)

    def ref_zd(in0, in1, s0, s1, imm2):
        a = (in0 - s0).astype(np.float32)
        return ((a * a).astype(np.float32) + in1).astype(np.float32)

    def ref_minmax(in0, in1, s0, s1, imm2):
        out = np.minimum(in0, in1).astype(np.float32)
        return out, out.max(axis=-1, keepdims=True)

    def ref_selsum(in0, in1, s0, s1, imm2):
        m = (in0 == s0)
        out = np.where(m, in1, np.float32(0.0)).astype(np.float32)
        return out, out.sum(axis=-1, keepdims=True, dtype=np.float32)

    specs = [
        # txy = (x-px)^2 + (y-py)^2
        ("PN_SQ2", Spec(body=sq(Src0 - C0) + sq(Src1 - C1), reference=ref_sq2),
         None),
        # d = (z-pz)^2 + txy
        ("PN_ZD", Spec(body=sq(Src0 - C0) + Src1, reference=ref_zd), None),
        # mind = min(mind, d); accum = rowmax(mind)
        ("PN_MINMAX", Spec(body=minn(Src0, Src1), accum=AluOp.MAX,
                           reference=ref_minmax), None),
        # out = (mind==gmax) ? coord : 0 ; accum = rowsum(out)
        ("PN_SELSUM", Spec(body=select(eq(Src0, C0), Src1, Zero),
                           accum=AluOp.ADD, reference=ref_selsum), None),
    ]

    made = {}
    for name, spec, _ in specs:
        op = DveOp(name, spec, subdim=False, uops_sha={})
        OPS.append(op)
        CUSTOM_DVE_SPECS[name] = spec
        _SUB_OPCODE_FOR_NAME[name] = max(_SUB_OPCODE_FOR_NAME.values()) + 1 \
            if _SUB_OPCODE_FOR_NAME else 1
        assert _SUB_OPCODE_FOR_NAME[name] < 0x20
        for ver in ("v3", "v4"):
            s = DveOpSpec(
                name=name,
                opcode=_SUB_OPCODE_FOR_NAME[name],
                uops=lower(spec, ver=ver),
                rd1_en=_has_src1(spec),
            )
            op.uops_sha[ver] = s.sha(ver)
        made[name] = op
    return made


# --------------------------------------------------------------------------
# Device program
# --------------------------------------------------------------------------
def _build_program(fps_iters=FPS_ITERS):
    import concourse.bacc as bacc
    import concourse.bass as bass
    import concourse.mybir as mybir
    import concourse.bass_isa as bass_isa
    from concourse.tile import TileContext

    ops = _register_dve_ops()
    OP_SQ2, OP_ZD, OP_MINMAX, OP_SELSUM = (
        ops["PN_SQ2"], ops["PN_ZD"], ops["PN_MINMAX"], ops["PN_SELSUM"])

    f32 = mybir.dt.float32
    i16 = mybir.dt.int16
    i32 = mybir.dt.int32
    u32 = mybir.dt.uint32
    AF = mybir.ActivationFunctionType
    Alu = mybir.AluOpType
    Ax = mybir.AxisListType
    RO = bass_isa.ReduceOp

    nc = bacc.Bacc("TRN2", target_bir_lowering=False, debug=False,
                   enable_asserts=False, num_devices=1)

    # ---- I/O ----
    xp = nc.dram_tensor("xplane", [N], f32, kind="ExternalInput").ap()
    yp = nc.dram_tensor("yplane", [N], f32, kind="ExternalInput").ap()
    zp = nc.dram_tensor("zplane", [N], f32, kind="ExternalInput").ap()
    featd = nc.dram_tensor("feat", [IN_CH, N], f32, kind="ExternalInput").ap()
    w0d = nc.dram_tensor("w0T", [IN_CH, 128], f32, kind="ExternalInput").ap()
    w1d = nc.dram_tensor("w1T", [128, 128], f32, kind="ExternalInput").ap()
    w2d = nc.dram_tensor("w2T", [128, 256], f32, kind="ExternalInput").ap()
    bnsd = [nc.dram_tensor(f"bns{i}", [MLP_CH[i]], f32, kind="ExternalInput").ap()
            for i in range(3)]
    bnbd = [nc.dram_tensor(f"bnb{i}", [MLP_CH[i]], f32, kind="ExternalInput").ap()
            for i in range(3)]
    identd = nc.dram_tensor("ident", [128, 128], f32, kind="ExternalInput").ap()

    o_newxyz = nc.dram_tensor("o_newxyz", [3 * S], f32, kind="ExternalOutput").ap()
    o_feat = nc.dram_tensor("o_feat", [128, RPC * NCALLS * 64], f32,
                            kind="ExternalOutput").ap()

    # ---- DRAM staging ----
    gt_dram = nc.dram_tensor("gt_stage", [N, 256], f32, kind="Internal").ap()
    q2_dram = nc.dram_tensor("q2_stage", [N], f32, kind="Internal").ap()
    cm_dram = nc.dram_tensor("cm_stage", [3 * S], f32, kind="Internal").ap()
    nc2_dram = nc.dram_tensor("negc2_stage", [S], f32, kind="Internal").ap()


    from concourse import library_config

    with TileContext(nc) as tc:
        nc.gpsimd.load_library(library_config.mlp)
        with (
            tc.tile_pool(name="persist", bufs=1) as pp,
            tc.tile_pool(name="fps", bufs=2) as fp,
            tc.tile_pool(name="mlpconst", bufs=1) as mc,
            tc.tile_pool(name="mlp", bufs=3) as mp,
            tc.tile_pool(name="mlppsum", bufs=1, space="PSUM") as mpp,
            tc.tile_pool(name="g3pool", bufs=1) as gp,
            tc.tile_pool(name="gtpsum", bufs=2, space="PSUM") as gtp,
            tc.tile_pool(name="gtsb", bufs=3) as gts_pool,
        ):
            # ======== constants / FPS init ========
            xT = pp.tile([128, NB], f32, tag="xT")
            yT = pp.tile([128, NB], f32, tag="yT")
            zT = pp.tile([128, NB], f32, tag="zT")
            mind = pp.tile([128, NB], f32, tag="mind")
            nc.sync.dma_start(out=xT, in_=xp.rearrange("(p f) -> p f", p=128))
            nc.sync.dma_start(out=yT, in_=yp.rearrange("(p f) -> p f", p=128))
            nc.sync.dma_start(out=zT, in_=zp.rearrange("(p f) -> p f", p=128))
            nc.vector.memset(mind, 1e10)

            # q2 = x^2 + y^2 + z^2  (same op order as reference)
            q2t = pp.tile([128, NB], f32, tag="q2t")
            nc.vector._custom_dve(OP_SQ2, out=q2t, in0=xT, in1=yT, s0=0.0, s1=0.0)
            nc.vector._custom_dve(OP_ZD, out=q2t, in0=zT, in1=q2t, s0=0.0)
            nc.sync.dma_start(out=q2_dram.rearrange("(p f) -> p f", p=128), in_=q2t)

            # p0 = xyz[0] broadcast
            p0 = pp.tile([1, 3], f32, tag="p0")
            nc.sync.dma_start(out=p0[:, 0:1], in_=xp[0:1].rearrange("(p f) -> p f", p=1))
            nc.sync.dma_start(out=p0[:, 1:2], in_=yp[0:1].rearrange("(p f) -> p f", p=1))
            nc.sync.dma_start(out=p0[:, 2:3], in_=zp[0:1].rearrange("(p f) -> p f", p=1))

            pstore = pp.tile([1, 3 * S], f32, tag="pstore")
            cmaj2 = pp.tile([1, 3 * S], f32, tag="cmaj2")
            if fps_iters < FPS_ITERS:  # debug builds: zero unwritten slots
                nc.vector.memset(pstore, 0.0)
                nc.vector.memset(cmaj2, 0.0)
            cmaj3 = cmaj2.rearrange("p (c s) -> p c s", c=3)

            pbs = [pp.tile([128, 3], f32, tag=f"pb{i}", name=f"pb{i}")
                   for i in range(3)]
            nc.gpsimd.partition_broadcast(pbs[0], p0, channels=128)

            def store_point(pb, t):
                pb3 = pb[0:1, 0:3].rearrange("p (c s) -> p c s", s=1)
                nc.scalar.activation(out=pstore[:, 3 * t:3 * t + 3],
                                     in_=pb[0:1, 0:3], func=AF.Copy)
                nc.scalar.activation(out=cmaj3[:, :, t:t + 1], in_=pb3,
                                     func=AF.Copy, scale=2.0)

            store_point(pbs[0], 0)

            # ======== FPS loop ========
            for t in range(1, fps_iters + 1):
                pb_cur = pbs[(t - 1) % 3]
                pb_nxt = pbs[t % 3]
                txy = fp.tile([128, NB], f32, tag="txy")
                dd = fp.tile([128, NB], f32, tag="dd")
                rmax = fp.tile([128, 1], f32, tag="rmax")
                gmax = fp.tile([128, 1], f32, tag="gmax")
                sel3 = fp.tile([128, 3], f32, tag="sel3")
                trash = fp.tile([128, NB], f32, tag="trash")

                nc.vector._custom_dve(OP_SQ2, out=txy, in0=xT, in1=yT,
                                      s0=pb_cur[:, 0:1], s1=pb_cur[:, 1:2])
                nc.vector._custom_dve(OP_ZD, out=dd, in0=zT, in1=txy,
                                      s0=pb_cur[:, 2:3])
                nc.vector._custom_dve(OP_MINMAX, out=mind, in0=mind, in1=dd,
                                      accum_out=rmax)
                nc.gpsimd.partition_all_reduce(gmax, rmax, channels=128,
                                               reduce_op=RO.max)
                nc.vector._custom_dve(OP_SELSUM, out=trash, in0=mind, in1=xT,
                                      s0=gmax[:, 0:1], accum_out=sel3[:, 0:1])
                nc.vector._custom_dve(OP_SELSUM, out=trash, in0=mind, in1=yT,
                                      s0=gmax[:, 0:1], accum_out=sel3[:, 1:2])
                nc.vector._custom_dve(OP_SELSUM, out=trash, in0=mind, in1=zT,
                                      s0=gmax[:, 0:1], accum_out=sel3[:, 2:3])
                nc.gpsimd.partition_all_reduce(pb_nxt, sel3, channels=128,
                                               reduce_op=RO.add)
                store_point(pb_nxt, t)

            # flush centroid buffers to DRAM
            nc.sync.dma_start(out=o_newxyz.rearrange("(p f) -> p f", p=1),
                              in_=pstore)
            nc.sync.dma_start(out=cm_dram.rearrange("(p f) -> p f", p=1),
                              in_=cmaj2)

            # -c2 = -0.25 * ((2cx)^2 + (2cy)^2 + (2cz)^2)
            c2t = pp.tile([1, S], f32, tag="c2t")
            nc.vector._custom_dve(OP_SQ2, out=c2t, in0=cmaj2[:, 0:S],
                                  in1=cmaj2[:, S:2 * S], s0=0.0, s1=0.0)
            nc.vector._custom_dve(OP_ZD, out=c2t, in0=cmaj2[:, 2 * S:3 * S],
                                  in1=c2t, s0=0.0)
            negc2 = pp.tile([1, S], f32, tag="negc2")
            nc.scalar.activation(out=negc2, in_=c2t, func=AF.Copy, scale=-0.25)
            nc.sync.dma_start(out=nc2_dram.rearrange("(p f) -> p f", p=1),
                              in_=negc2)

            # ==== MLP over all N points (overlaps FPS via Tile scheduling) ==
            featS = mc.tile([IN_CH, N], f32, tag="featS")
            nc.sync.dma_start(out=featS, in_=featd)
            w0S = mc.tile([IN_CH, 128], f32, tag="w0S")
            w1S = mc.tile([128, 128], f32, tag="w1S")
            w2S = mc.tile([128, 256], f32, tag="w2S")
            nc.sync.dma_start(out=w0S, in_=w0d)
            nc.sync.dma_start(out=w1S, in_=w1d)
            nc.sync.dma_start(out=w2S, in_=w2d)
            identS = mc.tile([128, 128], f32, tag="identS")
            nc.sync.dma_start(out=identS, in_=identd)
            bns = []
            bnb = []
            for i in range(3):
                ch = min(MLP_CH[i], 128)
                nparts = MLP_CH[i] // ch
                st = mc.tile([ch, nparts], f32, tag=f"bns{i}")
                bt = mc.tile([ch, nparts], f32, tag=f"bnb{i}")
                nc.sync.dma_start(out=st, in_=bnsd[i].rearrange(
                    "(h p) -> p h", p=ch))
                nc.sync.dma_start(out=bt, in_=bnbd[i].rearrange(
                    "(h p) -> p h", p=ch))
                bns.append(st)
                bnb.append(bt)

            g3a = gp.tile([128, N], f32, tag="g3a")
            g3b = gp.tile([128, N], f32, tag="g3b")

            CH = 512
            for n in range(N // CH):
                sl = slice(n * CH, (n + 1) * CH)
                ps1 = mpp.tile([128, CH], f32, tag="ps1")
                a1 = mp.tile([128, CH], f32, tag="a1")
                nc.tensor.matmul(ps1, w0S, featS[:, sl], start=True, stop=True)
                nc.scalar.activation(out=a1, in_=ps1, func=AF.Relu,
                                     scale=bns[0][:, 0:1], bias=bnb[0][:, 0:1])
                ps2 = mpp.tile([128, CH], f32, tag="ps2")
                a2 = mp.tile([128, CH], f32, tag="a2")
                nc.tensor.matmul(ps2, w1S, a1, start=True, stop=True)
                nc.scalar.activation(out=a2, in_=ps2, func=AF.Relu,
                                     scale=bns[1][:, 0:1], bias=bnb[1][:, 0:1])
                ps3 = mpp.tile([128, CH], f32, tag="ps3")
                nc.tensor.matmul(ps3, w2S[:, 0:128], a2, start=True, stop=True)
                nc.scalar.activation(out=g3a[:, sl], in_=ps3, func=AF.Relu,
                                     scale=bns[2][:, 0:1], bias=bnb[2][:, 0:1])
                ps4 = mpp.tile([128, CH], f32, tag="ps4")
                nc.tensor.matmul(ps4, w2S[:, 128:256], a2, start=True, stop=True)
                nc.scalar.activation(out=g3b[:, sl], in_=ps4, func=AF.Relu,
                                     scale=bns[2][:, 1:2], bias=bnb[2][:, 1:2])

            # transpose G [256, N] -> GT [N, 256] and stage to DRAM
            gt2 = gt_dram  # [N, 256]
            for n2 in range(N // 128):
                sl = slice(n2 * 128, (n2 + 1) * 128)
                pta = gtp.tile([128, 128], f32, tag="pta")
                ptb = gtp.tile([128, 128], f32, tag="ptb")
                gts = gts_pool.tile([128, 256], f32, tag="gts")
                nc.tensor.transpose(pta, g3a[:, sl], identS)
                nc.tensor.transpose(ptb, g3b[:, sl], identS)
                nc.scalar.activation(out=gts[:, 0:128], in_=pta, func=AF.Copy)
                nc.scalar.activation(out=gts[:, 128:256], in_=ptb, func=AF.Copy)
                nc.sync.dma_start(out=gt2[sl, :], in_=gts)

        # ======== kNN: scores + exact top-32 per centroid ========
        with (
            tc.tile_pool(name="knnc", bufs=1) as kc,
            tc.tile_pool(name="knnpsum", bufs=2, space="PSUM") as kpp,
            tc.tile_pool(name="knns", bufs=2) as kp,
            tc.tile_pool(name="idxp", bufs=1) as tip,
            tc.tile_pool(name="gat", bufs=4) as gap,
            tc.tile_pool(name="mpo", bufs=1) as mpo,
        ):
            rhs5 = kc.tile([5, N], f32, tag="rhs5")
            nc.sync.dma_start(out=rhs5[0:1, :], in_=xp.rearrange("(p f) -> p f", p=1))
            nc.sync.dma_start(out=rhs5[1:2, :], in_=yp.rearrange("(p f) -> p f", p=1))
            nc.sync.dma_start(out=rhs5[2:3, :], in_=zp.rearrange("(p f) -> p f", p=1))
            nc.sync.dma_start(out=rhs5[3:4, :], in_=q2_dram.rearrange("(p f) -> p f", p=1))
            onesrow = kc.tile([1, N], f32, tag="onesrow")
            nc.vector.memset(onesrow, 1.0)
            nc.sync.dma_start(out=rhs5[4:5, :], in_=onesrow)

            lhsT5 = kc.tile([5, S], f32, tag="lhsT5")
            nc.sync.dma_start(out=lhsT5[0:3, :],
                              in_=cm_dram.rearrange("(c s) -> c s", c=3))
            negrow = kc.tile([1, S], f32, tag="negrow")
            nc.vector.memset(negrow, -1.0)
            nc.sync.dma_start(out=lhsT5[3:4, :], in_=negrow)
            nc.sync.dma_start(out=lhsT5[4:5, :],
                              in_=nc2_dram.rearrange("(p f) -> p f", p=1))

            acc = mpo.tile([128, SBLK * 256], f32, tag="acc")

            CH = 512
            for m in range(SBLK):
                sd = kp.tile([128, N], f32, tag="sd")
                for n in range(N // CH):
                    sl = slice(n * CH, (n + 1) * CH)
                    psS = kpp.tile([128, CH], f32, tag="psS")
                    nc.tensor.matmul(psS, lhsT5[:, m * 128:(m + 1) * 128],
                                     rhs5[:, sl], start=True, stop=True)
                    nc.scalar.activation(out=sd[:, sl], in_=psS, func=AF.Copy)

                idxi = tip.tile([128, 32], i32, tag="idxi", name=f"idxi{m}")
                for r in range(4):
                    wv = kp.tile([128, 8], f32, tag="wv")
                    wi = kp.tile([128, 8], u32, tag="wi")
                    nc.vector.max(out=wv, in_=sd)
                    nc.vector.max_index(out=wi, in_max=wv, in_values=sd)
                    if r < 3:
                        nc.vector.match_replace(out=sd, in_to_replace=wv,
                                                in_values=sd, imm_value=IMM_NEG)
                    nc.vector.tensor_copy(idxi[:, r * 8:(r + 1) * 8], wi)

                # gather the 32 neighbor feature rows per centroid, max-pool
                accm = acc[:, m * 256:(m + 1) * 256]
                for r in range(K):
                    gat = gap.tile([128, 256], f32, tag="gat")
                    nc.gpsimd.indirect_dma_start(
                        out=gat, out_offset=None, in_=gt_dram,
                        in_offset=bass.IndirectOffsetOnAxis(
                            ap=idxi[:, r:r + 1], axis=0))
                    if r == 0:
                        nc.vector.tensor_copy(accm, gat)
                    else:
                        nc.vector.tensor_tensor(out=accm, in0=accm, in1=gat,
                                                op=Alu.max)
            nc.sync.dma_start(out=o_feat, in_=acc)

    nc.compile()
    return nc


def _get_program(fps_iters=FPS_ITERS):
    key = fps_iters
    if key not in _BUILD_CACHE:
        _BUILD_CACHE[key] = _build_program(fps_iters)
    return _BUILD_CACHE[key]


# --------------------------------------------------------------------------
# Host wrapper
# --------------------------------------------------------------------------
def _host_inputs(b, xyz, feature, ws, bss):
    f32 = np.float32
    xb = np.ascontiguousarray(xyz[b, :, 0], f32)
    yb = np.ascontiguousarray(xyz[b, :, 1], f32)
    zb = np.ascontiguousarray(xyz[b, :, 2], f32)
    m = {
        "xplane": xb, "yplane": yb, "zplane": zb,
        "feat": np.ascontiguousarray(feature[b], f32),
        "w0T": np.ascontiguousarray(ws[0].T, f32),
        "w1T": np.ascontiguousarray(ws[1].T, f32),
        "w2T": np.ascontiguousarray(ws[2].T, f32),
        "ident": np.eye(128, dtype=f32),
    }
    for i, (bias, gamma, beta, mean, var) in enumerate(bss):
        inv = (gamma / np.sqrt(var + np.float32(EPS))).astype(f32)
        m[f"bns{i}"] = inv
        m[f"bnb{i}"] = ((bias - mean) * inv + beta).astype(f32)
    return m


def kernel(xyz, feature,
           w0, b0, gamma0, beta0, mean0, var0,
           w1, b1, gamma1, beta1, mean1, var1,
           w2, b2, gamma2, beta2, mean2, var2):
    from concourse.bass_utils import run_bass_kernel_spmd

    xyz = np.asarray(xyz, np.float32)
    feature = np.asarray(feature, np.float32)
    ws = [np.asarray(w, np.float32) for w in (w0, w1, w2)]
    bss = [
        (np.asarray(b0, np.float32), np.asarray(gamma0, np.float32),
         np.asarray(beta0, np.float32), np.asarray(mean0, np.float32),
         np.asarray(var0, np.float32)),
        (np.asarray(b1, np.float32), np.asarray(gamma1, np.float32),
         np.asarray(beta1, np.float32), np.asarray(mean1, np.float32),
         np.asarray(var1, np.float32)),
        (np.asarray(b2, np.float32), np.asarray(gamma2, np.float32),
         np.asarray(beta2, np.float32), np.asarray(mean2, np.float32),
         np.asarray(var2, np.float32)),
    ]

    nc = _get_program()
    in_maps = [_host_inputs(b, xyz, feature, ws, bss) for b in range(B)]
    res = run_bass_kernel_spmd(nc, in_maps, core_ids=list(range(B)))

    new_xyz = np.empty((B, S, 3), np.float32)
    new_feat = np.empty((B, 256, S), np.float32)
    for b in range(B):
        out = res.results[b]
        new_xyz[b] = out["o_newxyz"].reshape(S, 3)
        ft = out["o_feat"].reshape(128, 8, 256)
        new_feat[b] = np.transpose(ft, (2, 1, 0)).reshape(256, S)
    return new_xyz, new_feat


# revision 25
# speedup vs baseline: 1.2403x; 1.2403x over previous
"""PointNet Set Abstraction on 8 Trainium2 NeuronCores.

Sharding: data-parallel over batch B=8, one point cloud per core.

Per-core device pipeline:
  - FPS (1023 serial iterations) on DVE (custom fused ops) + gpsimd
    partition reduces. Produces the 1024 sampled points directly (no
    index extraction needed).
  - Pointwise MLP (3x conv1x1+BN+ReLU) over all N=8192 input points on
    PE/ACT (overlapped with FPS), transposed and staged to DRAM as
    GT [8192, 256].
  - kNN scores via K=5 PE matmul (s = 2 c.q - |q|^2 - |c|^2, monotone
    in -distance), exact top-32 per centroid row via vector max8 /
    max_index / match_replace rounds (ties resolved to lowest index,
    matching lax.top_k).
  - Feature gather: dma_gather of 4x8192 rows of 1KB from GT, max-pool
    over the 32 neighbors with strided tensor_reduce.

Host only reshapes/transposes per-core outputs into the full result.
"""

import numpy as np

B, N, S, K = 8, 8192, 1024, 32
IN_CH = 64
MLP_CH = [128, 128, 256]
EPS = 1e-5
FPS_ITERS = S - 1  # serial FPS steps after seeding index 0
NB = N // 128  # 64 free elems per partition in [128, 64] point tiles
SBLK = S // 128  # 8 centroid blocks
NCALLS = 4  # dma_gather calls (8 neighbor slots each)
RPC = K // NCALLS  # 8 r-slots per call
IMM_NEG = -3.0e38
PE_EXTRACT = True  # cross-partition FPS reduce via PE instead of gpsimd

_BUILD_CACHE = {}


# --------------------------------------------------------------------------
# Custom DVE ops (registered at runtime; self-pinned shas)
# --------------------------------------------------------------------------
def _register_dve_ops():
    import concourse.dve_ops as dve_ops
    from concourse.dve_ops import DveOp, OPS, CUSTOM_DVE_SPECS, _SUB_OPCODE_FOR_NAME
    from concourse.dve_spec import (
        Spec, Src0, Src1, C0, C1, Zero, AluOp, sq, minn, select, eq, lower,
        _has_src1,
    )
    from concourse.dve_uop import DveOpSpec

    if "PN_SQ2" in _SUB_OPCODE_FOR_NAME:
        from concourse.dve_ops import OPS as _O
        return {op.name: op for op in _O if op.name.startswith("PN_")}

    def ref_sq2(in0, in1, s0, s1, imm2):
        a = (in0 - s0).astype(np.float32)
        b = (in1 - s1).astype(np.float32)
        return (a * a + b * b).astype(np.float32)

    def ref_zd(in0, in1, s0, s1, imm2):
        a = (in0 - s0).astype(np.float32)
        return ((a * a).astype(np.float32) + in1).astype(np.float32)

    def ref_minmax(in0, in1, s0, s1, imm2):
        out = np.minimum(in0, in1).astype(np.float32)
        return out, out.max(axis=-1, keepdims=True)

    def ref_selsum(in0, in1, s0, s1, imm2):
        m = (in0 == s0)
        out = np.where(m, in1, np.float32(0.0)).astype(np.float32)
        return out, out.sum(axis=-1, keepdims=True, dtype=np.float32)

    specs = [
        # txy = (x-px)^2 + (y-py)^2
        ("PN_SQ2", Spec(body=sq(Src0 - C0) + sq(Src1 - C1), reference=ref_sq2),
         None),
        # d = (z-pz)^2 + txy
        ("PN_ZD", Spec(body=sq(Src0 - C0) + Src1, reference=ref_zd), None),
        # mind = min(mind, d); accum = rowmax(mind)
        ("PN_MINMAX", Spec(body=minn(Src0, Src1), accum=AluOp.MAX,
                           reference=ref_minmax), None),
        # out = (mind==gmax) ? coord : 0 ; accum = rowsum(out)
        ("PN_SELSUM", Spec(body=select(eq(Src0, C0), Src1, Zero),
                           accum=AluOp.ADD, reference=ref_selsum), None),
    ]

    made = {}
    for name, spec, _ in specs:
        op = DveOp(name, spec, subdim=False, uops_sha={})
        OPS.append(op)
        CUSTOM_DVE_SPECS[name] = spec
        _SUB_OPCODE_FOR_NAME[name] = max(_SUB_OPCODE_FOR_NAME.values()) + 1 \
            if _SUB_OPCODE_FOR_NAME else 1
        assert _SUB_OPCODE_FOR_NAME[name] < 0x20
        for ver in ("v3", "v4"):
            s = DveOpSpec(
                name=name,
                opcode=_SUB_OPCODE_FOR_NAME[name],
                uops=lower(spec, ver=ver),
                rd1_en=_has_src1(spec),
            )
            op.uops_sha[ver] = s.sha(ver)
        made[name] = op
    return made


# --------------------------------------------------------------------------
# Device program
# --------------------------------------------------------------------------
def _build_program(fps_iters=FPS_ITERS):
    import concourse.bacc as bacc
    import concourse.bass as bass
    import concourse.mybir as mybir
    import concourse.bass_isa as bass_isa
    from concourse.tile import TileContext

    ops = _register_dve_ops()
    OP_SQ2, OP_ZD, OP_MINMAX, OP_SELSUM = (
        ops["PN_SQ2"], ops["PN_ZD"], ops["PN_MINMAX"], ops["PN_SELSUM"])

    f32 = mybir.dt.float32
    i16 = mybir.dt.int16
    i32 = mybir.dt.int32
    u32 = mybir.dt.uint32
    AF = mybir.ActivationFunctionType
    Alu = mybir.AluOpType
    Ax = mybir.AxisListType
    RO = bass_isa.ReduceOp

    nc = bacc.Bacc("TRN2", target_bir_lowering=False, debug=False,
                   enable_asserts=False, num_devices=1)

    # ---- I/O ----
    xp = nc.dram_tensor("xplane", [N], f32, kind="ExternalInput").ap()
    yp = nc.dram_tensor("yplane", [N], f32, kind="ExternalInput").ap()
    zp = nc.dram_tensor("zplane", [N], f32, kind="ExternalInput").ap()
    featd = nc.dram_tensor("feat", [IN_CH, N], f32, kind="ExternalInput").ap()
    w0d = nc.dram_tensor("w0T", [IN_CH, 128], f32, kind="ExternalInput").ap()
    w1d = nc.dram_tensor("w1T", [128, 128], f32, kind="ExternalInput").ap()
    w2d = nc.dram_tensor("w2T", [128, 256], f32, kind="ExternalInput").ap()
    bnsd = [nc.dram_tensor(f"bns{i}", [MLP_CH[i]], f32, kind="ExternalInput").ap()
            for i in range(3)]
    bnbd = [nc.dram_tensor(f"bnb{i}", [MLP_CH[i]], f32, kind="ExternalInput").ap()
            for i in range(3)]
    identd = nc.dram_tensor("ident", [128, 128], f32, kind="ExternalInput").ap()

    o_newxyz = nc.dram_tensor("o_newxyz", [3 * S], f32, kind="ExternalOutput").ap()
    o_feat = nc.dram_tensor("o_feat", [128, RPC * NCALLS * 64], f32,
                            kind="ExternalOutput").ap()

    # ---- DRAM staging ----
    gt_dram = nc.dram_tensor("gt_stage", [N, 256], f32, kind="Internal").ap()
    q2_dram = nc.dram_tensor("q2_stage", [N], f32, kind="Internal").ap()
    cm_dram = nc.dram_tensor("cm_stage", [3 * S], f32, kind="Internal").ap()
    nc2_dram = nc.dram_tensor("negc2_stage", [S], f32, kind="Internal").ap()


    from concourse import library_config

    with TileContext(nc) as tc:
        nc.gpsimd.load_library(library_config.mlp)
        with (
            tc.tile_pool(name="persist", bufs=1) as pp,
            tc.tile_pool(name="fps", bufs=2) as fp,
            tc.tile_pool(name="fpsp", bufs=2, space="PSUM") as fpp,
            tc.tile_pool(name="mlpconst", bufs=1) as mc,
            tc.tile_pool(name="mlp", bufs=3) as mp,
            tc.tile_pool(name="mlppsum", bufs=1, space="PSUM") as mpp,
            tc.tile_pool(name="g3pool", bufs=1) as gp,
            tc.tile_pool(name="gtpsum", bufs=1, space="PSUM") as gtp,
            tc.tile_pool(name="gtsb", bufs=3) as gts_pool,
        ):
            # ======== constants / FPS init ========
            xT = pp.tile([128, NB], f32, tag="xT")
            yT = pp.tile([128, NB], f32, tag="yT")
            zT = pp.tile([128, NB], f32, tag="zT")
            mind = pp.tile([128, NB], f32, tag="mind")
            nc.sync.dma_start(out=xT, in_=xp.rearrange("(p f) -> p f", p=128))
            nc.sync.dma_start(out=yT, in_=yp.rearrange("(p f) -> p f", p=128))
            nc.sync.dma_start(out=zT, in_=zp.rearrange("(p f) -> p f", p=128))
            nc.vector.memset(mind, 1e10)

            # q2 = x^2 + y^2 + z^2  (same op order as reference)
            q2t = pp.tile([128, NB], f32, tag="q2t")
            nc.vector._custom_dve(OP_SQ2, out=q2t, in0=xT, in1=yT, s0=0.0, s1=0.0)
            nc.vector._custom_dve(OP_ZD, out=q2t, in0=zT, in1=q2t, s0=0.0)
            nc.sync.dma_start(out=q2_dram.rearrange("(p f) -> p f", p=128), in_=q2t)

            # p0 = xyz[0] broadcast
            p0 = pp.tile([1, 3], f32, tag="p0")
            nc.sync.dma_start(out=p0[:, 0:1], in_=xp[0:1].rearrange("(p f) -> p f", p=1))
            nc.sync.dma_start(out=p0[:, 1:2], in_=yp[0:1].rearrange("(p f) -> p f", p=1))
            nc.sync.dma_start(out=p0[:, 2:3], in_=zp[0:1].rearrange("(p f) -> p f", p=1))

            pstore = pp.tile([1, 3 * S], f32, tag="pstore")
            cmaj2 = pp.tile([1, 3 * S], f32, tag="cmaj2")
            if fps_iters < FPS_ITERS:  # debug builds: zero unwritten slots
                nc.vector.memset(pstore, 0.0)
                nc.vector.memset(cmaj2, 0.0)
            cmaj3 = cmaj2.rearrange("p (c s) -> p c s", c=3)

            def store_point(src13, t):
                src3 = src13[0:1, 0:3].rearrange("p (c s) -> p c s", s=1)
                nc.scalar.activation(out=pstore[:, 3 * t:3 * t + 3],
                                     in_=src13[0:1, 0:3], func=AF.Copy)
                nc.scalar.activation(out=cmaj3[:, :, t:t + 1], in_=src3,
                                     func=AF.Copy, scale=2.0)

            store_point(p0, 0)

            if PE_EXTRACT:
                identF = pp.tile([128, 128], f32, tag="identF")
                nc.sync.dma_start(out=identF, in_=identd)
                ones_row = pp.tile([1, 128], f32, tag="ones_row")
                nc.vector.memset(ones_row, 1.0)
                ones_col = pp.tile([128, 1], f32, tag="ones_col")
                nc.vector.memset(ones_col, 1.0)
                ps0 = fpp.tile([128, 160], f32, tag="fpsps", name="ps0")
                nc.tensor.matmul(ps0[:, 136:139], ones_row, p0,
                                 start=True, stop=True)
                pb_cur = ps0[:, 136:139]
            else:
                pbs = [pp.tile([128, 3], f32, tag=f"pb{i}", name=f"pb{i}")
                       for i in range(3)]
                nc.gpsimd.partition_broadcast(pbs[0], p0, channels=128)
                pb_cur = pbs[0]

            # ======== FPS loop ========
            for t in range(1, fps_iters + 1):
                txy = fp.tile([128, NB], f32, tag="txy")
                dd = fp.tile([128, NB], f32, tag="dd")
                rmax = fp.tile([128, 1], f32, tag="rmax")
                sel3 = fp.tile([128, 3], f32, tag="sel3")
                trash = fp.tile([128, NB], f32, tag="trash")

                nc.vector._custom_dve(OP_SQ2, out=txy, in0=xT, in1=yT,
                                      s0=pb_cur[:, 0:1], s1=pb_cur[:, 1:2])
                nc.vector._custom_dve(OP_ZD, out=dd, in0=zT, in1=txy,
                                      s0=pb_cur[:, 2:3])
                nc.vector._custom_dve(OP_MINMAX, out=mind, in0=mind, in1=dd,
                                      accum_out=rmax)
                if PE_EXTRACT:
                    ps = fpp.tile([128, 160], f32, tag="fpsps",
                                  name=f"ps{t}")
                    nc.tensor.transpose(ps[0:1, 0:128], rmax, identF)
                    gm = fp.tile([1, 1], f32, tag="gm")
                    nc.vector.tensor_reduce(out=gm, in_=ps[0:1, 0:128],
                                            axis=Ax.X, op=Alu.max)
                    nc.tensor.matmul(ps[:, 128:129], ones_row, gm,
                                     start=True, stop=True)
                    gm_b = ps[:, 128:129]
                    nc.vector._custom_dve(OP_SELSUM, out=trash, in0=mind, in1=xT,
                                          s0=gm_b, accum_out=sel3[:, 0:1])
                    nc.vector._custom_dve(OP_SELSUM, out=trash, in0=mind, in1=yT,
                                          s0=gm_b, accum_out=sel3[:, 1:2])
                    nc.vector._custom_dve(OP_SELSUM, out=trash, in0=mind, in1=zT,
                                          s0=gm_b, accum_out=sel3[:, 2:3])
                    nc.tensor.matmul(ps[0:1, 132:135], ones_col, sel3,
                                     start=True, stop=True)
                    pr = fp.tile([1, 3], f32, tag="pr")
                    nc.vector.tensor_copy(pr, ps[0:1, 132:135])
                    nc.tensor.matmul(ps[:, 136:139], ones_row, pr,
                                     start=True, stop=True)
                    store_point(pr, t)
                    pb_cur = ps[:, 136:139]
                else:
                    gmax = fp.tile([128, 1], f32, tag="gmax")
                    nc.gpsimd.partition_all_reduce(gmax, rmax, channels=128,
                                                   reduce_op=RO.max)
                    nc.vector._custom_dve(OP_SELSUM, out=trash, in0=mind, in1=xT,
                                          s0=gmax[:, 0:1], accum_out=sel3[:, 0:1])
                    nc.vector._custom_dve(OP_SELSUM, out=trash, in0=mind, in1=yT,
                                          s0=gmax[:, 0:1], accum_out=sel3[:, 1:2])
                    nc.vector._custom_dve(OP_SELSUM, out=trash, in0=mind, in1=zT,
                                          s0=gmax[:, 0:1], accum_out=sel3[:, 2:3])
                    pb_nxt = pbs[t % 3]
                    nc.gpsimd.partition_all_reduce(pb_nxt, sel3, channels=128,
                                                   reduce_op=RO.add)
                    store_point(pb_nxt, t)
                    pb_cur = pb_nxt

            # flush centroid buffers to DRAM
            nc.sync.dma_start(out=o_newxyz.rearrange("(p f) -> p f", p=1),
                              in_=pstore)
            nc.sync.dma_start(out=cm_dram.rearrange("(p f) -> p f", p=1),
                              in_=cmaj2)

            # -c2 = -0.25 * ((2cx)^2 + (2cy)^2 + (2cz)^2)
            c2t = pp.tile([1, S], f32, tag="c2t")
            nc.vector._custom_dve(OP_SQ2, out=c2t, in0=cmaj2[:, 0:S],
                                  in1=cmaj2[:, S:2 * S], s0=0.0, s1=0.0)
            nc.vector._custom_dve(OP_ZD, out=c2t, in0=cmaj2[:, 2 * S:3 * S],
                                  in1=c2t, s0=0.0)
            negc2 = pp.tile([1, S], f32, tag="negc2")
            nc.scalar.activation(out=negc2, in_=c2t, func=AF.Copy, scale=-0.25)
            nc.sync.dma_start(out=nc2_dram.rearrange("(p f) -> p f", p=1),
                              in_=negc2)

            # ==== MLP over all N points (overlaps FPS via Tile scheduling) ==
            featS = mc.tile([IN_CH, N], f32, tag="featS")
            nc.sync.dma_start(out=featS, in_=featd)
            w0S = mc.tile([IN_CH, 128], f32, tag="w0S")
            w1S = mc.tile([128, 128], f32, tag="w1S")
            w2S = mc.tile([128, 256], f32, tag="w2S")
            nc.sync.dma_start(out=w0S, in_=w0d)
            nc.sync.dma_start(out=w1S, in_=w1d)
            nc.sync.dma_start(out=w2S, in_=w2d)
            identS = mc.tile([128, 128], f32, tag="identS")
            nc.sync.dma_start(out=identS, in_=identd)
            bns = []
            bnb = []
            for i in range(3):
                ch = min(MLP_CH[i], 128)
                nparts = MLP_CH[i] // ch
                st = mc.tile([ch, nparts], f32, tag=f"bns{i}")
                bt = mc.tile([ch, nparts], f32, tag=f"bnb{i}")
                nc.sync.dma_start(out=st, in_=bnsd[i].rearrange(
                    "(h p) -> p h", p=ch))
                nc.sync.dma_start(out=bt, in_=bnbd[i].rearrange(
                    "(h p) -> p h", p=ch))
                bns.append(st)
                bnb.append(bt)

            g3a = gp.tile([128, N], f32, tag="g3a")
            g3b = gp.tile([128, N], f32, tag="g3b")

            CH = 512
            for n in range(N // CH):
                sl = slice(n * CH, (n + 1) * CH)
                ps1 = mpp.tile([128, CH], f32, tag="ps1")
                a1 = mp.tile([128, CH], f32, tag="a1")
                nc.tensor.matmul(ps1, w0S, featS[:, sl], start=True, stop=True)
                nc.scalar.activation(out=a1, in_=ps1, func=AF.Relu,
                                     scale=bns[0][:, 0:1], bias=bnb[0][:, 0:1])
                ps2 = mpp.tile([128, CH], f32, tag="ps2")
                a2 = mp.tile([128, CH], f32, tag="a2")
                nc.tensor.matmul(ps2, w1S, a1, start=True, stop=True)
                nc.scalar.activation(out=a2, in_=ps2, func=AF.Relu,
                                     scale=bns[1][:, 0:1], bias=bnb[1][:, 0:1])
                ps3 = mpp.tile([128, CH], f32, tag="ps3")
                nc.tensor.matmul(ps3, w2S[:, 0:128], a2, start=True, stop=True)
                nc.scalar.activation(out=g3a[:, sl], in_=ps3, func=AF.Relu,
                                     scale=bns[2][:, 0:1], bias=bnb[2][:, 0:1])
                ps4 = mpp.tile([128, CH], f32, tag="ps4")
                nc.tensor.matmul(ps4, w2S[:, 128:256], a2, start=True, stop=True)
                nc.scalar.activation(out=g3b[:, sl], in_=ps4, func=AF.Relu,
                                     scale=bns[2][:, 1:2], bias=bnb[2][:, 1:2])

            # transpose G [256, N] -> GT [N, 256] and stage to DRAM
            gt2 = gt_dram  # [N, 256]
            for n2 in range(N // 128):
                sl = slice(n2 * 128, (n2 + 1) * 128)
                pta = gtp.tile([128, 128], f32, tag="pta")
                ptb = gtp.tile([128, 128], f32, tag="ptb")
                gts = gts_pool.tile([128, 256], f32, tag="gts")
                nc.tensor.transpose(pta, g3a[:, sl], identS)
                nc.tensor.transpose(ptb, g3b[:, sl], identS)
                nc.scalar.activation(out=gts[:, 0:128], in_=pta, func=AF.Copy)
                nc.scalar.activation(out=gts[:, 128:256], in_=ptb, func=AF.Copy)
                nc.sync.dma_start(out=gt2[sl, :], in_=gts)

        # ======== kNN: scores + exact top-32 per centroid ========
        with (
            tc.tile_pool(name="knnc", bufs=1) as kc,
            tc.tile_pool(name="knnpsum", bufs=2, space="PSUM") as kpp,
            tc.tile_pool(name="knns", bufs=2) as kp,
            tc.tile_pool(name="idxp", bufs=1) as tip,
            tc.tile_pool(name="gat", bufs=4) as gap,
            tc.tile_pool(name="mpo", bufs=1) as mpo,
        ):
            rhs5 = kc.tile([5, N], f32, tag="rhs5")
            nc.sync.dma_start(out=rhs5[0:1, :], in_=xp.rearrange("(p f) -> p f", p=1))
            nc.sync.dma_start(out=rhs5[1:2, :], in_=yp.rearrange("(p f) -> p f", p=1))
            nc.sync.dma_start(out=rhs5[2:3, :], in_=zp.rearrange("(p f) -> p f", p=1))
            nc.sync.dma_start(out=rhs5[3:4, :], in_=q2_dram.rearrange("(p f) -> p f", p=1))
            onesrow = kc.tile([1, N], f32, tag="onesrow")
            nc.vector.memset(onesrow, 1.0)
            nc.sync.dma_start(out=rhs5[4:5, :], in_=onesrow)

            lhsT5 = kc.tile([5, S], f32, tag="lhsT5")
            nc.sync.dma_start(out=lhsT5[0:3, :],
                              in_=cm_dram.rearrange("(c s) -> c s", c=3))
            negrow = kc.tile([1, S], f32, tag="negrow")
            nc.vector.memset(negrow, -1.0)
            nc.sync.dma_start(out=lhsT5[3:4, :], in_=negrow)
            nc.sync.dma_start(out=lhsT5[4:5, :],
                              in_=nc2_dram.rearrange("(p f) -> p f", p=1))

            acc = mpo.tile([128, SBLK * 256], f32, tag="acc")

            CH = 512
            for m in range(SBLK):
                sd = kp.tile([128, N], f32, tag="sd")
                for n in range(N // CH):
                    sl = slice(n * CH, (n + 1) * CH)
                    psS = kpp.tile([128, CH], f32, tag="psS")
                    nc.tensor.matmul(psS, lhsT5[:, m * 128:(m + 1) * 128],
                                     rhs5[:, sl], start=True, stop=True)
                    nc.scalar.activation(out=sd[:, sl], in_=psS, func=AF.Copy)

                idxi = tip.tile([128, 32], i32, tag="idxi", name=f"idxi{m}")
                for r in range(4):
                    wv = kp.tile([128, 8], f32, tag="wv")
                    wi = kp.tile([128, 8], u32, tag="wi")
                    nc.vector.max(out=wv, in_=sd)
                    nc.vector.max_index(out=wi, in_max=wv, in_values=sd)
                    if r < 3:
                        nc.vector.match_replace(out=sd, in_to_replace=wv,
                                                in_values=sd, imm_value=IMM_NEG)
                    nc.vector.tensor_copy(idxi[:, r * 8:(r + 1) * 8], wi)

                # gather the 32 neighbor feature rows per centroid, max-pool
                accm = acc[:, m * 256:(m + 1) * 256]
                for r in range(K):
                    gat = gap.tile([128, 256], f32, tag="gat")
                    nc.gpsimd.indirect_dma_start(
                        out=gat, out_offset=None, in_=gt_dram,
                        in_offset=bass.IndirectOffsetOnAxis(
                            ap=idxi[:, r:r + 1], axis=0))
                    if r == 0:
                        nc.vector.tensor_copy(accm, gat)
                    else:
                        nc.vector.tensor_tensor(out=accm, in0=accm, in1=gat,
                                                op=Alu.max)
            nc.sync.dma_start(out=o_feat, in_=acc)

    nc.compile()
    return nc


def _get_program(fps_iters=FPS_ITERS):
    key = fps_iters
    if key not in _BUILD_CACHE:
        _BUILD_CACHE[key] = _build_program(fps_iters)
    return _BUILD_CACHE[key]


# --------------------------------------------------------------------------
# Host wrapper
# --------------------------------------------------------------------------
def _host_inputs(b, xyz, feature, ws, bss):
    f32 = np.float32
    xb = np.ascontiguousarray(xyz[b, :, 0], f32)
    yb = np.ascontiguousarray(xyz[b, :, 1], f32)
    zb = np.ascontiguousarray(xyz[b, :, 2], f32)
    m = {
        "xplane": xb, "yplane": yb, "zplane": zb,
        "feat": np.ascontiguousarray(feature[b], f32),
        "w0T": np.ascontiguousarray(ws[0].T, f32),
        "w1T": np.ascontiguousarray(ws[1].T, f32),
        "w2T": np.ascontiguousarray(ws[2].T, f32),
        "ident": np.eye(128, dtype=f32),
    }
    for i, (bias, gamma, beta, mean, var) in enumerate(bss):
        inv = (gamma / np.sqrt(var + np.float32(EPS))).astype(f32)
        m[f"bns{i}"] = inv
        m[f"bnb{i}"] = ((bias - mean) * inv + beta).astype(f32)
    return m


def kernel(xyz, feature,
           w0, b0, gamma0, beta0, mean0, var0,
           w1, b1, gamma1, beta1, mean1, var1,
           w2, b2, gamma2, beta2, mean2, var2):
    from concourse.bass_utils import run_bass_kernel_spmd

    xyz = np.asarray(xyz, np.float32)
    feature = np.asarray(feature, np.float32)
    ws = [np.asarray(w, np.float32) for w in (w0, w1, w2)]
    bss = [
        (np.asarray(b0, np.float32), np.asarray(gamma0, np.float32),
         np.asarray(beta0, np.float32), np.asarray(mean0, np.float32),
         np.asarray(var0, np.float32)),
        (np.asarray(b1, np.float32), np.asarray(gamma1, np.float32),
         np.asarray(beta1, np.float32), np.asarray(mean1, np.float32),
         np.asarray(var1, np.float32)),
        (np.asarray(b2, np.float32), np.asarray(gamma2, np.float32),
         np.asarray(beta2, np.float32), np.asarray(mean2, np.float32),
         np.asarray(var2, np.float32)),
    ]

    nc = _get_program()
    in_maps = [_host_inputs(b, xyz, feature, ws, bss) for b in range(B)]
    res = run_bass_kernel_spmd(nc, in_maps, core_ids=list(range(B)))

    new_xyz = np.empty((B, S, 3), np.float32)
    new_feat = np.empty((B, 256, S), np.float32)
    for b in range(B):
        out = res.results[b]
        new_xyz[b] = out["o_newxyz"].reshape(S, 3)
        ft = out["o_feat"].reshape(128, 8, 256)
        new_feat[b] = np.transpose(ft, (2, 1, 0)).reshape(256, S)
    return new_xyz, new_feat


# revision 26
# speedup vs baseline: 1.5069x; 1.2149x over previous
"""PointNet Set Abstraction on 8 Trainium2 NeuronCores.

Sharding: data-parallel over batch B=8, one point cloud per core.

Per-core device pipeline:
  - FPS (1023 serial iterations) on DVE (custom fused ops) + gpsimd
    partition reduces. Produces the 1024 sampled points directly (no
    index extraction needed).
  - Pointwise MLP (3x conv1x1+BN+ReLU) over all N=8192 input points on
    PE/ACT (overlapped with FPS), transposed and staged to DRAM as
    GT [8192, 256].
  - kNN scores via K=5 PE matmul (s = 2 c.q - |q|^2 - |c|^2, monotone
    in -distance), exact top-32 per centroid row via vector max8 /
    max_index / match_replace rounds (ties resolved to lowest index,
    matching lax.top_k).
  - Feature gather: dma_gather of 4x8192 rows of 1KB from GT, max-pool
    over the 32 neighbors with strided tensor_reduce.

Host only reshapes/transposes per-core outputs into the full result.
"""

import numpy as np

B, N, S, K = 8, 8192, 1024, 32
IN_CH = 64
MLP_CH = [128, 128, 256]
EPS = 1e-5
FPS_ITERS = S - 1  # serial FPS steps after seeding index 0
NB = N // 128  # 64 free elems per partition in [128, 64] point tiles
SBLK = S // 128  # 8 centroid blocks
NCALLS = 4  # dma_gather calls (8 neighbor slots each)
RPC = K // NCALLS  # 8 r-slots per call
IMM_NEG = -3.0e38
PE_EXTRACT = False  # cross-partition FPS reduce via PE instead of gpsimd

_BUILD_CACHE = {}


# --------------------------------------------------------------------------
# Custom DVE ops (registered at runtime; self-pinned shas)
# --------------------------------------------------------------------------
def _register_dve_ops():
    import concourse.dve_ops as dve_ops
    from concourse.dve_ops import DveOp, OPS, CUSTOM_DVE_SPECS, _SUB_OPCODE_FOR_NAME
    from concourse.dve_spec import (
        Spec, Src0, Src1, C0, C1, Zero, AluOp, sq, minn, select, eq, lower,
        _has_src1,
    )
    from concourse.dve_uop import DveOpSpec

    if "PN_SQ2" in _SUB_OPCODE_FOR_NAME:
        from concourse.dve_ops import OPS as _O
        return {op.name: op for op in _O if op.name.startswith("PN_")}

    def ref_sq2(in0, in1, s0, s1, imm2):
        a = (in0 - s0).astype(np.float32)
        b = (in1 - s1).astype(np.float32)
        return (a * a + b * b).astype(np.float32)

    def ref_zd(in0, in1, s0, s1, imm2):
        a = (in0 - s0).astype(np.float32)
        return ((a * a).astype(np.float32) + in1).astype(np.float32)

    def ref_minmax(in0, in1, s0, s1, imm2):
        out = np.minimum(in0, in1).astype(np.float32)
        return out, out.max(axis=-1, keepdims=True)

    def ref_selsum(in0, in1, s0, s1, imm2):
        m = (in0 == s0)
        out = np.where(m, in1, np.float32(0.0)).astype(np.float32)
        return out, out.sum(axis=-1, keepdims=True, dtype=np.float32)

    specs = [
        # txy = (x-px)^2 + (y-py)^2
        ("PN_SQ2", Spec(body=sq(Src0 - C0) + sq(Src1 - C1), reference=ref_sq2),
         None),
        # d = (z-pz)^2 + txy
        ("PN_ZD", Spec(body=sq(Src0 - C0) + Src1, reference=ref_zd), None),
        # mind = min(mind, d); accum = rowmax(mind)
        ("PN_MINMAX", Spec(body=minn(Src0, Src1), accum=AluOp.MAX,
                           reference=ref_minmax), None),
        # out = (mind==gmax) ? coord : 0 ; accum = rowsum(out)
        ("PN_SELSUM", Spec(body=select(eq(Src0, C0), Src1, Zero),
                           accum=AluOp.ADD, reference=ref_selsum), None),
    ]

    made = {}
    for name, spec, _ in specs:
        op = DveOp(name, spec, subdim=False, uops_sha={})
        OPS.append(op)
        CUSTOM_DVE_SPECS[name] = spec
        _SUB_OPCODE_FOR_NAME[name] = max(_SUB_OPCODE_FOR_NAME.values()) + 1 \
            if _SUB_OPCODE_FOR_NAME else 1
        assert _SUB_OPCODE_FOR_NAME[name] < 0x20
        for ver in ("v3", "v4"):
            s = DveOpSpec(
                name=name,
                opcode=_SUB_OPCODE_FOR_NAME[name],
                uops=lower(spec, ver=ver),
                rd1_en=_has_src1(spec),
            )
            op.uops_sha[ver] = s.sha(ver)
        made[name] = op
    return made


# --------------------------------------------------------------------------
# Device program
# --------------------------------------------------------------------------
def _build_program(fps_iters=FPS_ITERS):
    import concourse.bacc as bacc
    import concourse.bass as bass
    import concourse.mybir as mybir
    import concourse.bass_isa as bass_isa
    from concourse.tile import TileContext

    ops = _register_dve_ops()
    OP_SQ2, OP_ZD, OP_MINMAX, OP_SELSUM = (
        ops["PN_SQ2"], ops["PN_ZD"], ops["PN_MINMAX"], ops["PN_SELSUM"])

    f32 = mybir.dt.float32
    i16 = mybir.dt.int16
    i32 = mybir.dt.int32
    u32 = mybir.dt.uint32
    AF = mybir.ActivationFunctionType
    Alu = mybir.AluOpType
    Ax = mybir.AxisListType
    RO = bass_isa.ReduceOp

    nc = bacc.Bacc("TRN2", target_bir_lowering=False, debug=False,
                   enable_asserts=False, num_devices=1)

    # ---- I/O ----
    xp = nc.dram_tensor("xplane", [N], f32, kind="ExternalInput").ap()
    yp = nc.dram_tensor("yplane", [N], f32, kind="ExternalInput").ap()
    zp = nc.dram_tensor("zplane", [N], f32, kind="ExternalInput").ap()
    featd = nc.dram_tensor("feat", [IN_CH, N], f32, kind="ExternalInput").ap()
    w0d = nc.dram_tensor("w0T", [IN_CH, 128], f32, kind="ExternalInput").ap()
    w1d = nc.dram_tensor("w1T", [128, 128], f32, kind="ExternalInput").ap()
    w2d = nc.dram_tensor("w2T", [128, 256], f32, kind="ExternalInput").ap()
    bnsd = [nc.dram_tensor(f"bns{i}", [MLP_CH[i]], f32, kind="ExternalInput").ap()
            for i in range(3)]
    bnbd = [nc.dram_tensor(f"bnb{i}", [MLP_CH[i]], f32, kind="ExternalInput").ap()
            for i in range(3)]
    identd = nc.dram_tensor("ident", [128, 128], f32, kind="ExternalInput").ap()

    o_newxyz = nc.dram_tensor("o_newxyz", [3 * S], f32, kind="ExternalOutput").ap()
    o_feat = nc.dram_tensor("o_feat", [128, RPC * NCALLS * 64], f32,
                            kind="ExternalOutput").ap()

    # ---- DRAM staging ----
    gt_dram = nc.dram_tensor("gt_stage", [N, 256], f32, kind="Internal").ap()
    q2_dram = nc.dram_tensor("q2_stage", [N], f32, kind="Internal").ap()
    cm_dram = nc.dram_tensor("cm_stage", [3 * S], f32, kind="Internal").ap()
    nc2_dram = nc.dram_tensor("negc2_stage", [S], f32, kind="Internal").ap()


    from concourse import library_config

    with TileContext(nc) as tc:
        nc.gpsimd.load_library(library_config.mlp)
        with (
            tc.tile_pool(name="persist", bufs=1) as pp,
            tc.tile_pool(name="fps", bufs=2) as fp,
            tc.tile_pool(name="fpsp", bufs=2, space="PSUM") as fpp,
            tc.tile_pool(name="mlpconst", bufs=1) as mc,
            tc.tile_pool(name="mlp", bufs=3) as mp,
            tc.tile_pool(name="mlppsum", bufs=1, space="PSUM") as mpp,
            tc.tile_pool(name="g3pool", bufs=1) as gp,
            tc.tile_pool(name="gtpsum", bufs=1, space="PSUM") as gtp,
            tc.tile_pool(name="gtsb", bufs=3) as gts_pool,
        ):
            # ======== constants / FPS init ========
            xT = pp.tile([128, NB], f32, tag="xT")
            yT = pp.tile([128, NB], f32, tag="yT")
            zT = pp.tile([128, NB], f32, tag="zT")
            mind = pp.tile([128, NB], f32, tag="mind")
            nc.sync.dma_start(out=xT, in_=xp.rearrange("(p f) -> p f", p=128))
            nc.sync.dma_start(out=yT, in_=yp.rearrange("(p f) -> p f", p=128))
            nc.sync.dma_start(out=zT, in_=zp.rearrange("(p f) -> p f", p=128))
            nc.vector.memset(mind, 1e10)

            # q2 = x^2 + y^2 + z^2  (same op order as reference)
            q2t = pp.tile([128, NB], f32, tag="q2t")
            nc.vector._custom_dve(OP_SQ2, out=q2t, in0=xT, in1=yT, s0=0.0, s1=0.0)
            nc.vector._custom_dve(OP_ZD, out=q2t, in0=zT, in1=q2t, s0=0.0)
            nc.sync.dma_start(out=q2_dram.rearrange("(p f) -> p f", p=128), in_=q2t)

            # p0 = xyz[0] broadcast
            p0 = pp.tile([1, 3], f32, tag="p0")
            nc.sync.dma_start(out=p0[:, 0:1], in_=xp[0:1].rearrange("(p f) -> p f", p=1))
            nc.sync.dma_start(out=p0[:, 1:2], in_=yp[0:1].rearrange("(p f) -> p f", p=1))
            nc.sync.dma_start(out=p0[:, 2:3], in_=zp[0:1].rearrange("(p f) -> p f", p=1))

            pstore = pp.tile([1, 3 * S], f32, tag="pstore")
            cmaj2 = pp.tile([1, 3 * S], f32, tag="cmaj2")
            if fps_iters < FPS_ITERS:  # debug builds: zero unwritten slots
                nc.vector.memset(pstore, 0.0)
                nc.vector.memset(cmaj2, 0.0)
            cmaj3 = cmaj2.rearrange("p (c s) -> p c s", c=3)

            def store_point(src13, t):
                src3 = src13[0:1, 0:3].rearrange("p (c s) -> p c s", s=1)
                nc.scalar.activation(out=pstore[:, 3 * t:3 * t + 3],
                                     in_=src13[0:1, 0:3], func=AF.Copy)
                nc.scalar.activation(out=cmaj3[:, :, t:t + 1], in_=src3,
                                     func=AF.Copy, scale=2.0)

            store_point(p0, 0)

            if PE_EXTRACT:
                identF = pp.tile([128, 128], f32, tag="identF")
                nc.sync.dma_start(out=identF, in_=identd)
                ones_row = pp.tile([1, 128], f32, tag="ones_row")
                nc.vector.memset(ones_row, 1.0)
                ones_col = pp.tile([128, 1], f32, tag="ones_col")
                nc.vector.memset(ones_col, 1.0)
                ps0 = fpp.tile([128, 160], f32, tag="fpsps", name="ps0")
                nc.tensor.matmul(ps0[:, 136:139], ones_row, p0,
                                 start=True, stop=True)
                pb_cur = ps0[:, 136:139]
            else:
                pbs = [pp.tile([128, 3], f32, tag=f"pb{i}", name=f"pb{i}")
                       for i in range(3)]
                nc.gpsimd.partition_broadcast(pbs[0], p0, channels=128)
                pb_cur = pbs[0]

            # ======== FPS loop ========
            for t in range(1, fps_iters + 1):
                txy = fp.tile([128, NB], f32, tag="txy")
                dd = fp.tile([128, NB], f32, tag="dd")
                rmax = fp.tile([128, 1], f32, tag="rmax")
                sel3 = fp.tile([128, 3], f32, tag="sel3")
                trash = fp.tile([128, NB], f32, tag="trash")

                nc.vector._custom_dve(OP_SQ2, out=txy, in0=xT, in1=yT,
                                      s0=pb_cur[:, 0:1], s1=pb_cur[:, 1:2])
                nc.vector._custom_dve(OP_ZD, out=dd, in0=zT, in1=txy,
                                      s0=pb_cur[:, 2:3])
                nc.vector._custom_dve(OP_MINMAX, out=mind, in0=mind, in1=dd,
                                      accum_out=rmax)
                if PE_EXTRACT:
                    ps = fpp.tile([128, 160], f32, tag="fpsps",
                                  name=f"ps{t}")
                    nc.tensor.transpose(ps[0:1, 0:128], rmax, identF)
                    gm = fp.tile([1, 1], f32, tag="gm")
                    nc.vector.tensor_reduce(out=gm, in_=ps[0:1, 0:128],
                                            axis=Ax.X, op=Alu.max)
                    nc.tensor.matmul(ps[:, 128:129], ones_row, gm,
                                     start=True, stop=True)
                    gm_b = ps[:, 128:129]
                    nc.vector._custom_dve(OP_SELSUM, out=trash, in0=mind, in1=xT,
                                          s0=gm_b, accum_out=sel3[:, 0:1])
                    nc.vector._custom_dve(OP_SELSUM, out=trash, in0=mind, in1=yT,
                                          s0=gm_b, accum_out=sel3[:, 1:2])
                    nc.vector._custom_dve(OP_SELSUM, out=trash, in0=mind, in1=zT,
                                          s0=gm_b, accum_out=sel3[:, 2:3])
                    nc.tensor.matmul(ps[0:1, 132:135], ones_col, sel3,
                                     start=True, stop=True)
                    pr = fp.tile([1, 3], f32, tag="pr")
                    nc.vector.tensor_copy(pr, ps[0:1, 132:135])
                    nc.tensor.matmul(ps[:, 136:139], ones_row, pr,
                                     start=True, stop=True)
                    store_point(pr, t)
                    pb_cur = ps[:, 136:139]
                else:
                    gmax = fp.tile([128, 1], f32, tag="gmax")
                    nc.gpsimd.partition_all_reduce(gmax, rmax, channels=128,
                                                   reduce_op=RO.max)
                    nc.vector._custom_dve(OP_SELSUM, out=trash, in0=mind, in1=xT,
                                          s0=gmax[:, 0:1], accum_out=sel3[:, 0:1])
                    nc.vector._custom_dve(OP_SELSUM, out=trash, in0=mind, in1=yT,
                                          s0=gmax[:, 0:1], accum_out=sel3[:, 1:2])
                    nc.vector._custom_dve(OP_SELSUM, out=trash, in0=mind, in1=zT,
                                          s0=gmax[:, 0:1], accum_out=sel3[:, 2:3])
                    pb_nxt = pbs[t % 3]
                    nc.gpsimd.partition_all_reduce(pb_nxt, sel3, channels=128,
                                                   reduce_op=RO.add)
                    store_point(pb_nxt, t)
                    pb_cur = pb_nxt

            # flush centroid buffers to DRAM
            nc.sync.dma_start(out=o_newxyz.rearrange("(p f) -> p f", p=1),
                              in_=pstore)
            nc.sync.dma_start(out=cm_dram.rearrange("(p f) -> p f", p=1),
                              in_=cmaj2)

            # -c2 = -0.25 * ((2cx)^2 + (2cy)^2 + (2cz)^2)
            c2t = pp.tile([1, S], f32, tag="c2t")
            nc.vector._custom_dve(OP_SQ2, out=c2t, in0=cmaj2[:, 0:S],
                                  in1=cmaj2[:, S:2 * S], s0=0.0, s1=0.0)
            nc.vector._custom_dve(OP_ZD, out=c2t, in0=cmaj2[:, 2 * S:3 * S],
                                  in1=c2t, s0=0.0)
            negc2 = pp.tile([1, S], f32, tag="negc2")
            nc.scalar.activation(out=negc2, in_=c2t, func=AF.Copy, scale=-0.25)
            nc.sync.dma_start(out=nc2_dram.rearrange("(p f) -> p f", p=1),
                              in_=negc2)

            # ==== MLP over all N points (overlaps FPS via Tile scheduling) ==
            featS = mc.tile([IN_CH, N], f32, tag="featS")
            nc.sync.dma_start(out=featS, in_=featd)
            w0S = mc.tile([IN_CH, 128], f32, tag="w0S")
            w1S = mc.tile([128, 128], f32, tag="w1S")
            w2S = mc.tile([128, 256], f32, tag="w2S")
            nc.sync.dma_start(out=w0S, in_=w0d)
            nc.sync.dma_start(out=w1S, in_=w1d)
            nc.sync.dma_start(out=w2S, in_=w2d)
            identS = mc.tile([128, 128], f32, tag="identS")
            nc.sync.dma_start(out=identS, in_=identd)
            bns = []
            bnb = []
            for i in range(3):
                ch = min(MLP_CH[i], 128)
                nparts = MLP_CH[i] // ch
                st = mc.tile([ch, nparts], f32, tag=f"bns{i}")
                bt = mc.tile([ch, nparts], f32, tag=f"bnb{i}")
                nc.sync.dma_start(out=st, in_=bnsd[i].rearrange(
                    "(h p) -> p h", p=ch))
                nc.sync.dma_start(out=bt, in_=bnbd[i].rearrange(
                    "(h p) -> p h", p=ch))
                bns.append(st)
                bnb.append(bt)

            g3a = gp.tile([128, N], f32, tag="g3a")
            g3b = gp.tile([128, N], f32, tag="g3b")

            CH = 512
            for n in range(N // CH):
                sl = slice(n * CH, (n + 1) * CH)
                ps1 = mpp.tile([128, CH], f32, tag="ps1")
                a1 = mp.tile([128, CH], f32, tag="a1")
                nc.tensor.matmul(ps1, w0S, featS[:, sl], start=True, stop=True)
                nc.scalar.activation(out=a1, in_=ps1, func=AF.Relu,
                                     scale=bns[0][:, 0:1], bias=bnb[0][:, 0:1])
                ps2 = mpp.tile([128, CH], f32, tag="ps2")
                a2 = mp.tile([128, CH], f32, tag="a2")
                nc.tensor.matmul(ps2, w1S, a1, start=True, stop=True)
                nc.scalar.activation(out=a2, in_=ps2, func=AF.Relu,
                                     scale=bns[1][:, 0:1], bias=bnb[1][:, 0:1])
                ps3 = mpp.tile([128, CH], f32, tag="ps3")
                nc.tensor.matmul(ps3, w2S[:, 0:128], a2, start=True, stop=True)
                nc.scalar.activation(out=g3a[:, sl], in_=ps3, func=AF.Relu,
                                     scale=bns[2][:, 0:1], bias=bnb[2][:, 0:1])
                ps4 = mpp.tile([128, CH], f32, tag="ps4")
                nc.tensor.matmul(ps4, w2S[:, 128:256], a2, start=True, stop=True)
                nc.scalar.activation(out=g3b[:, sl], in_=ps4, func=AF.Relu,
                                     scale=bns[2][:, 1:2], bias=bnb[2][:, 1:2])

            # transpose G [256, N] -> GT [N, 256] and stage to DRAM
            gt2 = gt_dram  # [N, 256]
            for n2 in range(N // 128):
                sl = slice(n2 * 128, (n2 + 1) * 128)
                pta = gtp.tile([128, 128], f32, tag="pta")
                ptb = gtp.tile([128, 128], f32, tag="ptb")
                gts = gts_pool.tile([128, 256], f32, tag="gts")
                nc.tensor.transpose(pta, g3a[:, sl], identS)
                nc.tensor.transpose(ptb, g3b[:, sl], identS)
                nc.scalar.activation(out=gts[:, 0:128], in_=pta, func=AF.Copy)
                nc.scalar.activation(out=gts[:, 128:256], in_=ptb, func=AF.Copy)
                nc.sync.dma_start(out=gt2[sl, :], in_=gts)

        # ======== kNN: scores + exact top-32 per centroid ========
        with (
            tc.tile_pool(name="knnc", bufs=1) as kc,
            tc.tile_pool(name="knnpsum", bufs=2, space="PSUM") as kpp,
            tc.tile_pool(name="knns", bufs=2) as kp,
            tc.tile_pool(name="idxp", bufs=1) as tip,
            tc.tile_pool(name="gat", bufs=4) as gap,
            tc.tile_pool(name="mpo", bufs=1) as mpo,
        ):
            rhs5 = kc.tile([5, N], f32, tag="rhs5")
            nc.sync.dma_start(out=rhs5[0:1, :], in_=xp.rearrange("(p f) -> p f", p=1))
            nc.sync.dma_start(out=rhs5[1:2, :], in_=yp.rearrange("(p f) -> p f", p=1))
            nc.sync.dma_start(out=rhs5[2:3, :], in_=zp.rearrange("(p f) -> p f", p=1))
            nc.sync.dma_start(out=rhs5[3:4, :], in_=q2_dram.rearrange("(p f) -> p f", p=1))
            onesrow = kc.tile([1, N], f32, tag="onesrow")
            nc.vector.memset(onesrow, 1.0)
            nc.sync.dma_start(out=rhs5[4:5, :], in_=onesrow)

            lhsT5 = kc.tile([5, S], f32, tag="lhsT5")
            nc.sync.dma_start(out=lhsT5[0:3, :],
                              in_=cm_dram.rearrange("(c s) -> c s", c=3))
            negrow = kc.tile([1, S], f32, tag="negrow")
            nc.vector.memset(negrow, -1.0)
            nc.sync.dma_start(out=lhsT5[3:4, :], in_=negrow)
            nc.sync.dma_start(out=lhsT5[4:5, :],
                              in_=nc2_dram.rearrange("(p f) -> p f", p=1))

            acc = mpo.tile([128, SBLK * 256], f32, tag="acc")

            CH = 512
            for m in range(SBLK):
                sd = kp.tile([128, N], f32, tag="sd")
                for n in range(N // CH):
                    sl = slice(n * CH, (n + 1) * CH)
                    psS = kpp.tile([128, CH], f32, tag="psS")
                    nc.tensor.matmul(psS, lhsT5[:, m * 128:(m + 1) * 128],
                                     rhs5[:, sl], start=True, stop=True)
                    nc.scalar.activation(out=sd[:, sl], in_=psS, func=AF.Copy)

                idxi = tip.tile([128, 32], i32, tag="idxi", name=f"idxi{m}")
                for r in range(4):
                    wv = kp.tile([128, 8], f32, tag="wv")
                    wi = kp.tile([128, 8], u32, tag="wi")
                    nc.vector.max(out=wv, in_=sd)
                    nc.vector.max_index(out=wi, in_max=wv, in_values=sd)
                    if r < 3:
                        nc.vector.match_replace(out=sd, in_to_replace=wv,
                                                in_values=sd, imm_value=IMM_NEG)
                    nc.vector.tensor_copy(idxi[:, r * 8:(r + 1) * 8], wi)

                # gather the 32 neighbor feature rows per centroid, max-pool
                accm = acc[:, m * 256:(m + 1) * 256]
                for r in range(K):
                    gat = gap.tile([128, 256], f32, tag="gat")
                    nc.gpsimd.indirect_dma_start(
                        out=gat, out_offset=None, in_=gt_dram,
                        in_offset=bass.IndirectOffsetOnAxis(
                            ap=idxi[:, r:r + 1], axis=0))
                    if r == 0:
                        nc.vector.tensor_copy(accm, gat)
                    else:
                        nc.vector.tensor_tensor(out=accm, in0=accm, in1=gat,
                                                op=Alu.max)
            nc.sync.dma_start(out=o_feat, in_=acc)

    nc.compile()
    return nc


def _get_program(fps_iters=FPS_ITERS):
    key = fps_iters
    if key not in _BUILD_CACHE:
        _BUILD_CACHE[key] = _build_program(fps_iters)
    return _BUILD_CACHE[key]


# --------------------------------------------------------------------------
# Host wrapper
# --------------------------------------------------------------------------
def _host_inputs(b, xyz, feature, ws, bss):
    f32 = np.float32
    xb = np.ascontiguousarray(xyz[b, :, 0], f32)
    yb = np.ascontiguousarray(xyz[b, :, 1], f32)
    zb = np.ascontiguousarray(xyz[b, :, 2], f32)
    m = {
        "xplane": xb, "yplane": yb, "zplane": zb,
        "feat": np.ascontiguousarray(feature[b], f32),
        "w0T": np.ascontiguousarray(ws[0].T, f32),
        "w1T": np.ascontiguousarray(ws[1].T, f32),
        "w2T": np.ascontiguousarray(ws[2].T, f32),
        "ident": np.eye(128, dtype=f32),
    }
    for i, (bias, gamma, beta, mean, var) in enumerate(bss):
        inv = (gamma / np.sqrt(var + np.float32(EPS))).astype(f32)
        m[f"bns{i}"] = inv
        m[f"bnb{i}"] = ((bias - mean) * inv + beta).astype(f32)
    return m


def kernel(xyz, feature,
           w0, b0, gamma0, beta0, mean0, var0,
           w1, b1, gamma1, beta1, mean1, var1,
           w2, b2, gamma2, beta2, mean2, var2):
    from concourse.bass_utils import run_bass_kernel_spmd

    xyz = np.asarray(xyz, np.float32)
    feature = np.asarray(feature, np.float32)
    ws = [np.asarray(w, np.float32) for w in (w0, w1, w2)]
    bss = [
        (np.asarray(b0, np.float32), np.asarray(gamma0, np.float32),
         np.asarray(beta0, np.float32), np.asarray(mean0, np.float32),
         np.asarray(var0, np.float32)),
        (np.asarray(b1, np.float32), np.asarray(gamma1, np.float32),
         np.asarray(beta1, np.float32), np.asarray(mean1, np.float32),
         np.asarray(var1, np.float32)),
        (np.asarray(b2, np.float32), np.asarray(gamma2, np.float32),
         np.asarray(beta2, np.float32), np.asarray(mean2, np.float32),
         np.asarray(var2, np.float32)),
    ]

    nc = _get_program()
    in_maps = [_host_inputs(b, xyz, feature, ws, bss) for b in range(B)]
    res = run_bass_kernel_spmd(nc, in_maps, core_ids=list(range(B)))

    new_xyz = np.empty((B, S, 3), np.float32)
    new_feat = np.empty((B, 256, S), np.float32)
    for b in range(B):
        out = res.results[b]
        new_xyz[b] = out["o_newxyz"].reshape(S, 3)
        ft = out["o_feat"].reshape(128, 8, 256)
        new_feat[b] = np.transpose(ft, (2, 1, 0)).reshape(256, S)
    return new_xyz, new_feat
